# revision 17
# baseline (speedup 1.0000x reference)
"""Trainium2 Bass kernel for dense multi-head attention (b=2, n=2048, dim=1024, h=16, dh=64).

Sharding: tensor-parallel over heads -- 2 heads per NeuronCore x 8 cores.
Each core computes QKV projection for its heads, attention, and a partial
output projection (w_out input-dim slice); partials are summed on host.

v2 redesign vs baseline:
  - S matmuls for the two heads are emitted back-to-back: head0's lhsT sits
    at base partition 0, head1's at 64, so tile_position row-tiling runs the
    two K=64 matmuls CONCURRENTLY in the PE array (~2x on the S phase).
  - Both heads' attention outputs are stacked into one [128, t] tile so the
    output projection contracts K=128 (full array) instead of 2 x K=64.
  - Window pipeline: 8 (batch, i-chunk) windows; window k computes S/exp/
    p-mult for chunk k, AV for chunk k-1, epilogue (1/s, normalize, out-proj)
    for chunk k-2.  K/V/Q projections are dripped into the windows as PE
    filler so the PE never idles long enough for the HAM clock gate to
    re-throttle it to 1.2 GHz (the baseline lost ~75us to this).
  - EBT (exp of attn bias, transposed) is relaid out on host so each
    window's [2048 j, 2 h, 512 i] block is one contiguous 32KB line per
    partition -> big DMA descriptors instead of 1KB ones.
  - 1/s is computed on [2,512] rows (both heads at once) and broadcast to
    all 128 partitions with a tiny K=2 matmul; the normalize multiply reads
    the broadcast directly from PSUM.
"""

import numpy as np
import ml_dtypes

import concourse.bass as bass
import concourse.tile as tile
from concourse import bacc
from concourse import mybir
from concourse.bass_utils import run_bass_kernel_spmd
from concourse.masks import make_identity

BF16 = mybir.dt.bfloat16
F32 = mybir.dt.float32
NPBF16 = ml_dtypes.bfloat16

B, N, DIM, HEADS, DH = 2, 2048, 1024, 16, 64
T = B * N  # 4096 tokens total
HPC = 2    # heads per core
NCORES = 8
SCALE = DH ** -0.5
EXP = mybir.ActivationFunctionType.Exp
LOG = mybir.ActivationFunctionType.Ln
MULT = mybir.AluOpType.mult


def _install_trace_hook():
    """Shim antenv.axon_hooks so run_bass_kernel_spmd(trace=True) can capture
    NTFF profiles via the axon .so (the agent image's antenv lacks the module)."""
    import sys
    import types

    try:
        import antenv

        if "antenv.axon_hooks" in sys.modules:
            return
        mod = types.ModuleType("antenv.axon_hooks")
        mod._HOOK = None
        mod.set_axon_ntff_profile_hook = lambda h: setattr(mod, "_HOOK", h)
        mod.get_axon_ntff_profile_hook = lambda: mod._HOOK
        sys.modules["antenv.axon_hooks"] = mod
        antenv.axon_hooks = mod
        from trn_agent_boot.trn_boot import _ntff_profile_via_ctypes

        hook = _ntff_profile_via_ctypes("/opt/axon/libaxon_pjrt.so")
        if hook is not None:
            mod._HOOK = hook
    except Exception:
        pass


_install_trace_hook()


class _OneActTableBacc(bacc.Bacc):
    """Force Exp and Ln onto the shared natural_log_exp_and_others ACT table
    set so the softmax exps and the 1/s=exp(-ln(s)) chain never thrash the
    ~1.3us ACT_TABLE_LOAD."""

    def insert_act_table_loads(self):
        import bass_rust as _bass_rust
        from concourse.hw_specs import get_activation_tables

        has_activation = any(
            isinstance(i, mybir.InstActivation)
            for b in self.main_func.blocks
            for i in b.instructions
        )
        if not has_activation:
            return
        tables = list(get_activation_tables(self.m.arch).items())
        shared = "natural_log_exp_and_others"
        strip = {
            mybir.ActivationFunctionType.Exp,
            mybir.ActivationFunctionType.Ln,
        }
        if any(nm == shared for nm, _ in tables):
            tables = [
                (nm, funcs if nm == shared else (funcs - strip))
                for nm, funcs in tables
            ]
        _bass_rust.insert_act_table_loads(self, tables)


def build_nc():
    nc = _OneActTableBacc()
    xs_d = nc.declare_dram_parameter("xs", [8, 128, 4096], BF16, isOutput=False)
    wT_d = nc.declare_dram_parameter("wT", [128, 8, 384], BF16, isOutput=False)
    woS_d = nc.declare_dram_parameter("woS", [128, 1024], BF16, isOutput=False)
    EBT_d = nc.declare_dram_parameter("EBT", [8, 128, 16384], BF16, isOutput=False)
    out_d = nc.declare_dram_parameter("out", [T, DIM], BF16, isOutput=True)

    with tile.TileContext(nc) as tc:
        with (
            tc.tile_pool(name="singles", bufs=1) as singles,
            tc.tile_pool(name="xs", bufs=2) as xspool,
            tc.tile_pool(name="ebt", bufs=2) as ebtpool,
            tc.tile_pool(name="pt", bufs=2) as ptpool,
            tc.tile_pool(name="p0", bufs=2) as p0pool,
            tc.tile_pool(name="ot", bufs=2) as otpool,
            tc.tile_pool(name="yt", bufs=2) as ytpool,
            tc.tile_pool(name="sr", bufs=1) as srpool,
            tc.tile_pool(name="sps", bufs=2, space="PSUM") as spspool,
            tc.tile_pool(name="av", bufs=1, space="PSUM") as avpool,
            tc.tile_pool(name="yps", bufs=1, space="PSUM") as ypspool,
        ):
            # ---- persistent SBUF ----
            wT_sb = singles.tile([128, 8, 384], BF16)
            woS_sb = singles.tile([128, 1024], BF16)
            ident = singles.tile([128, 128], BF16)
            qT_sb = singles.tile([128, T], BF16)   # rows: h0 q (64) | h1 q (64)
            kT_sb = singles.tile([128, T], BF16)
            # [j%128, bh, j//128, d|ones|pad]
            V_sb = singles.tile([128, B * HPC, 16, 128], BF16)
            ones_row = singles.tile([1, DH], BF16)  # bc lhsT (broadcast to 64 rows)

            make_identity(nc, ident)
            nc.vector.memset(V_sb, 0.0)
            nc.vector.memset(V_sb[:, :, :, DH : DH + 1], 1.0)
            nc.vector.memset(ones_row, 1.0)

            nc.sync.dma_start(out=wT_sb, in_=wT_d[:, :, :])
            nc.sync.dma_start(out=woS_sb, in_=woS_d[:, :])

            # ---------- projection pieces (PE filler work) ----------
            xs_tiles = {}

            def proj_load(tc8):
                """DMA one 512-token x chunk (shared by the q/k/v pieces)."""
                xs = xspool.tile([128, 4096], BF16, tag="xs")
                nc.sync.dma_start(out=xs, in_=xs_d[tc8])
                xs_tiles[tc8] = xs

            def proj_piece(eg, tc8):
                """One 512-token chunk of the q/k/v projection.
                eg: 0=q,1=k,2=v.  Uses the xs chunk loaded by proj_load."""
                xs = xs_tiles[tc8]
                ps = spspool.tile([128, 1024], F32, tag="sps")
                for dc in range(8):
                    nc.tensor.matmul(
                        ps[:, :512],
                        lhsT=wT_sb[:, dc, eg * 128 : (eg + 1) * 128],
                        rhs=xs[:, dc * 512 : (dc + 1) * 512],
                        start=(dc == 0),
                        stop=(dc == 7),
                    )
                if eg < 2:
                    dst = qT_sb if eg == 0 else kT_sb
                    nc.vector.tensor_copy(
                        out=dst[:, tc8 * 512 : (tc8 + 1) * 512], in_=ps[:, :512]
                    )
                else:
                    vstage = p0pool.tile([128, 512], BF16, tag="vstage")
                    nc.vector.tensor_copy(out=vstage, in_=ps[:, :512])
                    vtp = ypspool.tile([128, 512], BF16, tag="yps", name="vtp")
                    for tb in range(4):
                        nc.tensor.transpose(
                            vtp[:, tb * 128 : (tb + 1) * 128],
                            vstage[:, tb * 128 : (tb + 1) * 128],
                            ident,
                        )
                    b, jc0 = tc8 // 4, (tc8 % 4) * 4
                    nc.vector.tensor_copy(
                        out=V_sb[:, b * HPC : (b + 1) * HPC, jc0 : jc0 + 4, 0:DH],
                        in_=vtp.rearrange(
                            "p (tb h d) -> p h tb d", h=HPC, d=DH
                        ),
                    )

            # ---------- window state ----------
            pt_tiles = {}    # ck -> pT tile [128, 16, 1024]
            av_tiles = {}    # ck -> av tile [128, 2, 512] (h, i)
            ot_tiles = {}    # ck -> normalized OT tile [128, 512]

            def emit_S(ck, jp):
                b, ic = ck // 4, ck % 4
                sps = spspool.tile([128, 1024], F32, tag="sps")
                for h in range(2):
                    nc.tensor.matmul(
                        sps[:, h * 512 : (h + 1) * 512],
                        lhsT=kT_sb[
                            h * DH : (h + 1) * DH,
                            b * N + jp * 128 : b * N + (jp + 1) * 128,
                        ],
                        rhs=qT_sb[
                            h * DH : (h + 1) * DH,
                            b * N + ic * 512 : b * N + (ic + 1) * 512,
                        ],
                        start=True,
                        stop=True,
                    )
                return sps

            def emit_expmul(ck, jp, sps, ebt, pT):
                p0 = p0pool.tile([128, 1024], BF16, tag="p0")
                nc.scalar.activation(p0, sps, EXP)
                nc.vector.tensor_tensor(
                    pT[:, jp, :], p0, ebt[:, jp * 1024 : (jp + 1) * 1024], MULT
                )

            def emit_av(ck, k, av):
                """k-th of 32 AV matmuls for chunk ck (h-major: h = k // 16)."""
                b = ck // 4
                h, jb = k // 16, k % 16
                nc.tensor.matmul(
                    av[:, h, :],
                    lhsT=V_sb[:, b * HPC + h, jb, :],
                    rhs=pt_tiles[ck][:, jb, h * 512 : (h + 1) * 512],
                    start=(jb == 0),
                    stop=(jb == 15),
                )

            def emit_epi_a(ck):
                """OT copies + 1/s for chunk ck (av accumulation is complete)."""
                av = av_tiles[ck]
                ot = otpool.tile([128, 512], BF16, tag="otr", name="otraw")
                nc.vector.tensor_copy(out=ot[0:DH, :], in_=av[0:DH, 0, :])
                nc.vector.tensor_copy(out=ot[DH : 2 * DH, :], in_=av[0:DH, 1, :])
                s_sb = srpool.tile([1, 2, 512], F32, tag="ssb")
                nc.scalar.activation(s_sb, av[DH : DH + 1, :, :], LOG)
                r_sb = srpool.tile([1, 2, 512], BF16, tag="rsb")
                nc.scalar.activation(r_sb, s_sb, EXP, scale=-1.0)
                return (ck, av, ot, r_sb)

            def emit_epi_b(st):
                """Broadcast 1/s and normalize; returns out-proj thunks."""
                ck, av, ot, r_sb = st
                bc = ypspool.tile([128, 1024], F32, tag="yps", name="bc")
                nc.tensor.matmul(
                    bc[0:DH, 0:512],
                    lhsT=ones_row,
                    rhs=r_sb[:, 0, :],
                    start=True,
                    stop=True,
                )
                nc.tensor.matmul(
                    bc[DH : 2 * DH, 0:512],
                    lhsT=ones_row,
                    rhs=r_sb[:, 1, :],
                    start=True,
                    stop=True,
                )
                otn = otpool.tile([128, 512], BF16, tag="otn")
                nc.vector.tensor_tensor(otn, ot, bc[:, 0:512], MULT)
                ot_tiles[ck] = otn

                def mk(sub):
                    def thunk():
                        tt = ck * 4 + sub
                        yps = ypspool.tile([128, 1024], F32, tag="yps")
                        for dc2 in range(2):
                            nc.tensor.matmul(
                                yps[:, dc2 * 512 : (dc2 + 1) * 512],
                                lhsT=ot_tiles[ck][:, sub * 128 : (sub + 1) * 128],
                                rhs=woS_sb[:, dc2 * 512 : (dc2 + 1) * 512],
                                start=True,
                                stop=True,
                            )
                        yt = ytpool.tile([128, 1024], BF16, tag="yt")
                        nc.vector.tensor_copy(out=yt, in_=yps)
                        nc.gpsimd.dma_start(
                            out=out_d[tt * 128 : (tt + 1) * 128, :], in_=yt
                        )

                    return thunk

                return [mk(i) for i in range(4)]

            # ---------- filler schedule ----------
            # window -> list of pieces; "L<tc8>" = xs load, (eg, tc8) = proj
            fillers = {
                0: [("L", 0), (2, 0), ("L", 1), (2, 1), (0, 1), ("L", 2), (2, 2), ("L", 3), (2, 3)],
                1: [(0, 2), (0, 3)],
                2: [("L", 4), (1, 4), (0, 4), ("L", 5), (1, 5)],
                3: [("L", 6), (1, 6), ("L", 7), (1, 7), ("L", 4), (2, 4)],
                4: [("L", 5), (2, 5), (0, 5), ("L", 6), (2, 6)],
                5: [("L", 7), (2, 7), (0, 6)],
                6: [(0, 7)],
                7: [],
            }
            # jp slots where filler pieces are emitted (loads ride along)
            filler_slots = [2, 5, 7, 9, 11, 13]

            def emit_ebt_quarter(ck, q):
                if ck not in ebt_tiles:
                    ebt_tiles[ck] = ebtpool.tile(
                        [128, 16384], BF16, tag="ebt", name="ebt"
                    )
                nc.sync.dma_start(
                    out=ebt_tiles[ck][:, q * 4096 : (q + 1) * 4096],
                    in_=EBT_d[ck][:, q * 4096 : (q + 1) * 4096],
                )

            ebt_tiles = {}

            # ---------- prologue ----------
            for q in range(4):
                emit_ebt_quarter(0, q)
            proj_load(0)
            proj_piece(1, 0)    # k(b0) tc8 0
            proj_piece(0, 0)    # q(b0, 0)
            for tc8 in range(1, 4):
                proj_load(tc8)
                proj_piece(1, tc8)

            # ---------- windows ----------
            epi_b_queue = []
            outproj_queue = []

            for w in range(10):
                has_S = w < 8
                if has_S:
                    pT = ptpool.tile([128, 16, 1024], BF16, tag="pT")
                    pt_tiles[w] = pT
                if w >= 1 and (w - 1) < 8:
                    av_tiles[w - 1] = avpool.tile(
                        [128, 2, 512], F32, tag="av", name="av"
                    )
                av_k = 0
                fill = list(fillers.get(w, []))
                for jp in range(16):
                    if has_S:
                        sps = emit_S(w, jp)
                    if jp == 0 and epi_b_queue:
                        outproj_queue.extend(emit_epi_b(epi_b_queue.pop(0)))
                    if jp in (2, 5, 8, 11) and outproj_queue:
                        outproj_queue.pop(0)()
                    # AV drip for chunk w-1 (2 per jp)
                    if w >= 1 and (w - 1) < 8:
                        for _ in range(2):
                            if av_k < 32:
                                emit_av(w - 1, av_k, av_tiles[w - 1])
                                av_k += 1
                    # ebt prefetch for next window, one quarter at a time
                    if has_S and w + 1 < 8 and jp in (3, 6, 10, 14):
                        emit_ebt_quarter(w + 1, (3, 6, 10, 14).index(jp))
                    # projection filler
                    if has_S and jp in filler_slots and fill:
                        piece = fill.pop(0)
                        if piece[0] == "L":
                            proj_load(piece[1])
                            if fill and fill[0][0] != "L":
                                piece = fill.pop(0)
                                proj_piece(piece[0], piece[1])
                        else:
                            proj_piece(piece[0], piece[1])
                    if has_S:
                        emit_expmul(w, jp, sps, ebt_tiles[w], pT)
                # window end: flush fillers/AV, then free av(w-1) via epi_a
                while fill:
                    piece = fill.pop(0)
                    if piece[0] == "L":
                        proj_load(piece[1])
                    else:
                        proj_piece(piece[0], piece[1])
                if w >= 1 and (w - 1) < 8:
                    while av_k < 32:
                        emit_av(w - 1, av_k, av_tiles[w - 1])
                        av_k += 1
                    epi_b_queue.append(emit_epi_a(w - 1))
            # drain remaining epilogue work
            while epi_b_queue:
                outproj_queue.extend(emit_epi_b(epi_b_queue.pop(0)))
            for th in outproj_queue:
                th()

    return nc


_NC = None


def _get_nc():
    global _NC
    if _NC is None:
        _NC = build_nc()
        _NC.finalize()
    return _NC


def prepare_in_maps(x, mask, attn_bias, w_qkv, w_out, b_out):
    x = np.asarray(x, np.float32)
    mask = np.asarray(mask)
    attn_bias = np.asarray(attn_bias, np.float32)
    w_qkv = np.asarray(w_qkv, np.float32)
    w_out = np.asarray(w_out, np.float32)
    if not mask.all():
        attn_bias = np.where(mask[:, None, None, :], attn_bias, -np.inf)
    # exp(bias) transposed to [b, h, j, i]; masked -> 0 (multiplicative mask)
    EBT_full = np.exp(attn_bias).transpose(0, 1, 3, 2)
    # xs: [tc8, p, dc, i] = x[tc8*512+i, dc*128+p]
    x2 = x.reshape(T, DIM)
    xs = np.ascontiguousarray(
        x2.reshape(8, 512, 8, 128).transpose(0, 3, 2, 1).reshape(8, 128, 4096)
    ).astype(NPBF16)
    inner = HEADS * DH
    wq, wk, wv = w_qkv[:inner], w_qkv[inner : 2 * inner], w_qkv[2 * inner :]
    in_maps = []
    for c in range(NCORES):
        sl = slice(HPC * c * DH, HPC * (c + 1) * DH)
        wstack = np.concatenate([wq[sl] * SCALE, wk[sl], wv[sl]], axis=0)  # [384,1024]
        # wT: [p, dc, e] = wstack[e, dc*128+p]
        wT_c = np.ascontiguousarray(
            wstack.T.reshape(8, 128, 384).transpose(1, 0, 2)
        ).astype(NPBF16)
        woS = np.ascontiguousarray(w_out[:, sl].T).astype(NPBF16)  # [128, 1024]
        # EBT: [ck=b*4+ic, p, jb, h, il] = EBT_full[b, 2c+h, jb*128+p, ic*512+il]
        e = EBT_full[:, HPC * c : HPC * (c + 1)]  # [2, 2, 2048, 2048] (b,h,j,i)
        ebc = np.ascontiguousarray(
            e.reshape(2, 2, 16, 128, 4, 512)
            .transpose(0, 4, 3, 2, 1, 5)
            .reshape(8, 128, 16384)
        ).astype(NPBF16)
        in_maps.append({"xs": xs, "wT": wT_c, "woS": woS, "EBT": ebc})
    return in_maps


def run_device(in_maps, **kwargs):
    return run_bass_kernel_spmd(_get_nc(), in_maps, core_ids=list(range(NCORES)), **kwargs)


def finish(results, b_out):
    y = np.zeros((T, DIM), np.float32)
    for r in results:
        y += np.asarray(r["out"], np.float32)
    y += np.asarray(b_out, np.float32)[None, :]
    return y.reshape(B, N, DIM).astype(np.float32)


def kernel(x, mask, attn_bias, w_qkv, w_out, b_out):
    in_maps = prepare_in_maps(x, mask, attn_bias, w_qkv, w_out, b_out)
    res = run_device(in_maps)
    return finish(res.results, b_out)


# revision 20
# speedup vs baseline: 1.1950x; 1.1950x over previous
"""Trainium2 Bass kernel for dense multi-head attention (b=2, n=2048, dim=1024, h=16, dh=64).

Sharding: tensor-parallel over heads -- 2 heads per NeuronCore x 8 cores.
Each core computes QKV projection for its heads, attention, and a partial
output projection (w_out input-dim slice); partials are summed on host.

v2 redesign vs baseline:
  - S matmuls for the two heads are emitted back-to-back: head0's lhsT sits
    at base partition 0, head1's at 64, so tile_position row-tiling runs the
    two K=64 matmuls CONCURRENTLY in the PE array (~2x on the S phase).
  - Both heads' attention outputs are stacked into one [128, t] tile so the
    output projection contracts K=128 (full array) instead of 2 x K=64.
  - Window pipeline: 8 (batch, i-chunk) windows; window k computes S/exp/
    p-mult for chunk k, AV for chunk k-1, epilogue (1/s, normalize, out-proj)
    for chunk k-2.  K/V/Q projections are dripped into the windows as PE
    filler so the PE never idles long enough for the HAM clock gate to
    re-throttle it to 1.2 GHz (the baseline lost ~75us to this).
  - EBT (exp of attn bias, transposed) is relaid out on host so each
    window's [2048 j, 2 h, 512 i] block is one contiguous 32KB line per
    partition -> big DMA descriptors instead of 1KB ones.
  - 1/s is computed on [2,512] rows (both heads at once) and broadcast to
    all 128 partitions with a tiny K=2 matmul; the normalize multiply reads
    the broadcast directly from PSUM.
"""

import numpy as np
import ml_dtypes

import concourse.bass as bass
import concourse.tile as tile
from concourse import bacc
from concourse import mybir
from concourse.bass_utils import run_bass_kernel_spmd
from concourse.masks import make_identity

BF16 = mybir.dt.bfloat16
F32 = mybir.dt.float32
NPBF16 = ml_dtypes.bfloat16

B, N, DIM, HEADS, DH = 2, 2048, 1024, 16, 64
T = B * N  # 4096 tokens total
HPC = 2    # heads per core
NCORES = 8
SCALE = DH ** -0.5
EXP = mybir.ActivationFunctionType.Exp
LOG = mybir.ActivationFunctionType.Ln
MULT = mybir.AluOpType.mult


def _install_trace_hook():
    """Shim antenv.axon_hooks so run_bass_kernel_spmd(trace=True) can capture
    NTFF profiles via the axon .so (the agent image's antenv lacks the module)."""
    import sys
    import types

    try:
        import antenv

        if "antenv.axon_hooks" in sys.modules:
            return
        mod = types.ModuleType("antenv.axon_hooks")
        mod._HOOK = None
        mod.set_axon_ntff_profile_hook = lambda h: setattr(mod, "_HOOK", h)
        mod.get_axon_ntff_profile_hook = lambda: mod._HOOK
        sys.modules["antenv.axon_hooks"] = mod
        antenv.axon_hooks = mod
        from trn_agent_boot.trn_boot import _ntff_profile_via_ctypes

        hook = _ntff_profile_via_ctypes("/opt/axon/libaxon_pjrt.so")
        if hook is not None:
            mod._HOOK = hook
    except Exception:
        pass


_install_trace_hook()


class _OneActTableBacc(bacc.Bacc):
    """Force Exp and Ln onto the shared natural_log_exp_and_others ACT table
    set so the softmax exps and the 1/s=exp(-ln(s)) chain never thrash the
    ~1.3us ACT_TABLE_LOAD."""

    def insert_act_table_loads(self):
        import bass_rust as _bass_rust
        from concourse.hw_specs import get_activation_tables

        has_activation = any(
            isinstance(i, mybir.InstActivation)
            for b in self.main_func.blocks
            for i in b.instructions
        )
        if not has_activation:
            return
        tables = list(get_activation_tables(self.m.arch).items())
        shared = "natural_log_exp_and_others"
        strip = {
            mybir.ActivationFunctionType.Exp,
            mybir.ActivationFunctionType.Ln,
        }
        if any(nm == shared for nm, _ in tables):
            tables = [
                (nm, funcs if nm == shared else (funcs - strip))
                for nm, funcs in tables
            ]
        _bass_rust.insert_act_table_loads(self, tables)


def build_nc():
    nc = _OneActTableBacc()
    xs_d = nc.declare_dram_parameter("xs", [8, 128, 4096], BF16, isOutput=False)
    wT_d = nc.declare_dram_parameter("wT", [128, 8, 384], BF16, isOutput=False)
    woS_d = nc.declare_dram_parameter("woS", [128, 1024], BF16, isOutput=False)
    EBT_d = nc.declare_dram_parameter("EBT", [8, 128, 16384], BF16, isOutput=False)
    out_d = nc.declare_dram_parameter("out", [T, DIM], BF16, isOutput=True)

    with tile.TileContext(nc) as tc:
        with (
            tc.tile_pool(name="singles", bufs=1) as singles,
            tc.tile_pool(name="xs", bufs=2) as xspool,
            tc.tile_pool(name="ebt", bufs=2) as ebtpool,
            tc.tile_pool(name="pt", bufs=2) as ptpool,
            tc.tile_pool(name="p0", bufs=2) as p0pool,
            tc.tile_pool(name="ot", bufs=2) as otpool,
            tc.tile_pool(name="yt", bufs=2) as ytpool,
            tc.tile_pool(name="sr", bufs=1) as srpool,
            tc.tile_pool(name="sps", bufs=2, space="PSUM") as spspool,
            tc.tile_pool(name="av", bufs=1, space="PSUM") as avpool,
            tc.tile_pool(name="yps", bufs=1, space="PSUM") as ypspool,
        ):
            # ---- persistent SBUF ----
            wT_sb = singles.tile([128, 8, 384], BF16)
            woS_sb = singles.tile([128, 1024], BF16)
            ident = singles.tile([128, 128], BF16)
            qT_sb = singles.tile([128, T], BF16)   # rows: h0 q (64) | h1 q (64)
            kT_sb = singles.tile([128, T], BF16)
            # [j%128, bh, j//128, d|ones|pad]
            V_sb = singles.tile([128, B * HPC, 16, 128], BF16)
            ones_row = singles.tile([1, DH], BF16)  # bc lhsT (broadcast to 64 rows)

            make_identity(nc, ident)
            nc.vector.memset(V_sb, 0.0)
            nc.vector.memset(V_sb[:, :, :, DH : DH + 1], 1.0)
            nc.vector.memset(ones_row, 1.0)

            nc.sync.dma_start(out=wT_sb, in_=wT_d[:, :, :])
            nc.sync.dma_start(out=woS_sb, in_=woS_d[:, :])

            # ---------- projection pieces (PE filler work) ----------
            xs_tiles = {}

            def proj_load(tc8):
                """DMA one 512-token x chunk (shared by the q/k/v pieces)."""
                xs = xspool.tile([128, 4096], BF16, tag="xs")
                nc.sync.dma_start(out=xs, in_=xs_d[tc8])
                xs_tiles[tc8] = xs

            def proj_piece(eg, tc8):
                """One 512-token chunk of the q/k/v projection.
                eg: 0=q,1=k,2=v.  Uses the xs chunk loaded by proj_load."""
                xs = xs_tiles[tc8]
                ps = spspool.tile([128, 1024], F32, tag="sps")
                for dc in range(8):
                    nc.tensor.matmul(
                        ps[:, :512],
                        lhsT=wT_sb[:, dc, eg * 128 : (eg + 1) * 128],
                        rhs=xs[:, dc * 512 : (dc + 1) * 512],
                        start=(dc == 0),
                        stop=(dc == 7),
                    )
                if eg < 2:
                    dst = qT_sb if eg == 0 else kT_sb
                    nc.vector.tensor_copy(
                        out=dst[:, tc8 * 512 : (tc8 + 1) * 512], in_=ps[:, :512]
                    )
                else:
                    vstage = p0pool.tile([128, 512], BF16, tag="vstage")
                    nc.vector.tensor_copy(out=vstage, in_=ps[:, :512])
                    vtp = ypspool.tile([128, 512], BF16, tag="yps", name="vtp")
                    for tb in range(4):
                        nc.tensor.transpose(
                            vtp[:, tb * 128 : (tb + 1) * 128],
                            vstage[:, tb * 128 : (tb + 1) * 128],
                            ident,
                        )
                    b, jc0 = tc8 // 4, (tc8 % 4) * 4
                    nc.vector.tensor_copy(
                        out=V_sb[:, b * HPC : (b + 1) * HPC, jc0 : jc0 + 4, 0:DH],
                        in_=vtp.rearrange(
                            "p (tb h d) -> p h tb d", h=HPC, d=DH
                        ),
                    )

            # ---------- window state ----------
            pt_tiles = {}    # ck -> pT tile [128, 16, 1024]
            av_tiles = {}    # ck -> av tile [128, 2, 512] (h, i)
            ot_tiles = {}    # ck -> normalized OT tile [128, 512]

            def emit_S(ck, jp):
                b, ic = ck // 4, ck % 4
                sps = spspool.tile([128, 1024], F32, tag="sps")
                for h in range(2):
                    nc.tensor.matmul(
                        sps[:, h * 512 : (h + 1) * 512],
                        lhsT=kT_sb[
                            h * DH : (h + 1) * DH,
                            b * N + jp * 128 : b * N + (jp + 1) * 128,
                        ],
                        rhs=qT_sb[
                            h * DH : (h + 1) * DH,
                            b * N + ic * 512 : b * N + (ic + 1) * 512,
                        ],
                        start=True,
                        stop=True,
                    )
                return sps

            def emit_expmul(ck, jp, sps, ebt, pT):
                p0 = p0pool.tile([128, 1024], BF16, tag="p0")
                nc.scalar.activation(p0, sps, EXP)
                nc.vector.tensor_tensor(
                    pT[:, jp, :], p0, ebt[:, jp * 1024 : (jp + 1) * 1024], MULT
                )

            def emit_av(ck, k, av):
                """k-th of 32 AV matmuls for chunk ck (h-major: h = k // 16)."""
                b = ck // 4
                h, jb = k // 16, k % 16
                nc.tensor.matmul(
                    av[:, h, :],
                    lhsT=V_sb[:, b * HPC + h, jb, :],
                    rhs=pt_tiles[ck][:, jb, h * 512 : (h + 1) * 512],
                    start=(jb == 0),
                    stop=(jb == 15),
                )

            def emit_epi_a(ck):
                """OT copies + 1/s for chunk ck (av accumulation is complete)."""
                av = av_tiles[ck]
                ot = otpool.tile([128, 512], BF16, tag="otr", name="otraw")
                nc.vector.tensor_copy(out=ot[0:DH, :], in_=av[0:DH, 0, :])
                nc.vector.tensor_copy(out=ot[DH : 2 * DH, :], in_=av[0:DH, 1, :])
                s_sb = srpool.tile([1, 2, 512], F32, tag="ssb")
                nc.scalar.activation(s_sb, av[DH : DH + 1, :, :], LOG)
                r_sb = srpool.tile([1, 2, 512], BF16, tag="rsb")
                nc.scalar.activation(r_sb, s_sb, EXP, scale=-1.0)
                return (ck, av, ot, r_sb)

            def emit_epi_b(st):
                """Broadcast 1/s and normalize; returns out-proj thunks."""
                ck, av, ot, r_sb = st
                bc = ypspool.tile([128, 1024], F32, tag="yps", name="bc")
                nc.tensor.matmul(
                    bc[0:DH, 0:512],
                    lhsT=ones_row,
                    rhs=r_sb[:, 0, :],
                    start=True,
                    stop=True,
                )
                nc.tensor.matmul(
                    bc[DH : 2 * DH, 0:512],
                    lhsT=ones_row,
                    rhs=r_sb[:, 1, :],
                    start=True,
                    stop=True,
                )
                otn = otpool.tile([128, 512], BF16, tag="otn")
                nc.vector.tensor_tensor(otn, ot, bc[:, 0:512], MULT)
                ot_tiles[ck] = otn

                def mk(sub):
                    def thunk():
                        tt = ck * 4 + sub
                        yps = ypspool.tile([128, 1024], F32, tag="yps")
                        for dc2 in range(2):
                            nc.tensor.matmul(
                                yps[:, dc2 * 512 : (dc2 + 1) * 512],
                                lhsT=ot_tiles[ck][:, sub * 128 : (sub + 1) * 128],
                                rhs=woS_sb[:, dc2 * 512 : (dc2 + 1) * 512],
                                start=True,
                                stop=True,
                            )
                        yt = ytpool.tile([128, 1024], BF16, tag="yt")
                        nc.vector.tensor_copy(out=yt, in_=yps)
                        nc.gpsimd.dma_start(
                            out=out_d[tt * 128 : (tt + 1) * 128, :], in_=yt
                        )

                    return thunk

                return [mk(i) for i in range(4)]

            # ---------- filler schedule ----------
            # window -> list of pieces; "L<tc8>" = xs load, (eg, tc8) = proj
            # Each window gets a list of slot-groups; loads ("L", tc8) are
            # issued >=2 slots before the piece that consumes them so the
            # in-order PE queue never stalls behind an in-flight xs DMA.
            fillers = {
                0: [[("L", 0)], [("L", 1)], [(2, 0)], [(2, 1), ("L", 2)],
                    [(0, 1), ("L", 3)], [(2, 2)], [(2, 3)]],
                1: [[(0, 2)], [(0, 3)], [("L", 4)], [("L", 5)], [(1, 4)],
                    [(1, 5)], [(0, 4), ("L", 6)]],
                2: [[(1, 6)], [("L", 7)], [(1, 7)], [("L", 4)], [(2, 4)],
                    [("L", 5)], []],
                3: [[(2, 5)], [(0, 5)], [("L", 6)], [(2, 6)], [("L", 7)],
                    [], [(2, 7)]],
                4: [[(0, 6)]],
                5: [[(0, 7)]],
                6: [],
                7: [],
            }
            filler_slots = [1, 3, 5, 7, 9, 11, 13]

            def emit_ebt_quarter(ck, q):
                if ck not in ebt_tiles:
                    ebt_tiles[ck] = ebtpool.tile(
                        [128, 16384], BF16, tag="ebt", name="ebt"
                    )
                nc.sync.dma_start(
                    out=ebt_tiles[ck][:, q * 4096 : (q + 1) * 4096],
                    in_=EBT_d[ck][:, q * 4096 : (q + 1) * 4096],
                )

            ebt_tiles = {}

            # ---------- prologue ----------
            for q in range(4):
                emit_ebt_quarter(0, q)
            proj_load(0)
            proj_load(1)
            proj_piece(1, 0)    # k(b0) tc8 0
            proj_piece(0, 0)    # q(b0, 0)
            proj_load(2)
            proj_piece(1, 1)
            proj_load(3)
            proj_piece(1, 2)
            proj_piece(1, 3)

            # ---------- windows ----------
            epi_b_queue = []
            outproj_queue = []

            for w in range(10):
                has_S = w < 8
                if has_S:
                    pT = ptpool.tile([128, 16, 1024], BF16, tag="pT")
                    pt_tiles[w] = pT
                if w >= 1 and (w - 1) < 8:
                    av_tiles[w - 1] = avpool.tile(
                        [128, 2, 512], F32, tag="av", name="av"
                    )
                AVN = [0, 2, 2, 2, 2, 2, 2, 2, 2, 2, 2, 2, 2, 2, 3, 3]
                av_k = 0
                fill = list(fillers.get(w, []))
                for jp in range(16):
                    if has_S:
                        sps = emit_S(w, jp)
                    if jp == 0 and epi_b_queue:
                        outproj_queue.extend(emit_epi_b(epi_b_queue.pop(0)))
                    if jp in (4, 8, 11, 14) and outproj_queue:
                        outproj_queue.pop(0)()
                    # AV drip for chunk w-1
                    if w >= 1 and (w - 1) < 8:
                        for _ in range(AVN[jp]):
                            if av_k < 32:
                                emit_av(w - 1, av_k, av_tiles[w - 1])
                                av_k += 1
                    # ebt prefetch for next window, one quarter at a time
                    if has_S and w + 1 < 8 and jp in (3, 6, 10, 14):
                        emit_ebt_quarter(w + 1, (3, 6, 10, 14).index(jp))
                    # projection filler
                    if has_S and jp in filler_slots:
                        si = filler_slots.index(jp)
                        if si < len(fill):
                            for piece in fill[si]:
                                if piece[0] == "L":
                                    proj_load(piece[1])
                                else:
                                    proj_piece(piece[0], piece[1])
                    if has_S:
                        emit_expmul(w, jp, sps, ebt_tiles[w], pT)
                if w >= 1 and (w - 1) < 8:
                    while av_k < 32:
                        emit_av(w - 1, av_k, av_tiles[w - 1])
                        av_k += 1
                    epi_b_queue.append(emit_epi_a(w - 1))
            # drain remaining epilogue work
            while epi_b_queue:
                outproj_queue.extend(emit_epi_b(epi_b_queue.pop(0)))
            for th in outproj_queue:
                th()

    return nc


_NC = None


def _get_nc():
    global _NC
    if _NC is None:
        _NC = build_nc()
        _NC.finalize()
    return _NC


def prepare_in_maps(x, mask, attn_bias, w_qkv, w_out, b_out):
    x = np.asarray(x, np.float32)
    mask = np.asarray(mask)
    attn_bias = np.asarray(attn_bias, np.float32)
    w_qkv = np.asarray(w_qkv, np.float32)
    w_out = np.asarray(w_out, np.float32)
    if not mask.all():
        attn_bias = np.where(mask[:, None, None, :], attn_bias, -np.inf)
    # exp(bias) transposed to [b, h, j, i]; masked -> 0 (multiplicative mask)
    EBT_full = np.exp(attn_bias).transpose(0, 1, 3, 2)
    # xs: [tc8, p, dc, i] = x[tc8*512+i, dc*128+p]
    x2 = x.reshape(T, DIM)
    xs = np.ascontiguousarray(
        x2.reshape(8, 512, 8, 128).transpose(0, 3, 2, 1).reshape(8, 128, 4096)
    ).astype(NPBF16)
    inner = HEADS * DH
    wq, wk, wv = w_qkv[:inner], w_qkv[inner : 2 * inner], w_qkv[2 * inner :]
    in_maps = []
    for c in range(NCORES):
        sl = slice(HPC * c * DH, HPC * (c + 1) * DH)
        wstack = np.concatenate([wq[sl] * SCALE, wk[sl], wv[sl]], axis=0)  # [384,1024]
        # wT: [p, dc, e] = wstack[e, dc*128+p]
        wT_c = np.ascontiguousarray(
            wstack.T.reshape(8, 128, 384).transpose(1, 0, 2)
        ).astype(NPBF16)
        woS = np.ascontiguousarray(w_out[:, sl].T).astype(NPBF16)  # [128, 1024]
        # EBT: [ck=b*4+ic, p, jb, h, il] = EBT_full[b, 2c+h, jb*128+p, ic*512+il]
        e = EBT_full[:, HPC * c : HPC * (c + 1)]  # [2, 2, 2048, 2048] (b,h,j,i)
        ebc = np.ascontiguousarray(
            e.reshape(2, 2, 16, 128, 4, 512)
            .transpose(0, 4, 3, 2, 1, 5)
            .reshape(8, 128, 16384)
        ).astype(NPBF16)
        in_maps.append({"xs": xs, "wT": wT_c, "woS": woS, "EBT": ebc})
    return in_maps


def run_device(in_maps, **kwargs):
    return run_bass_kernel_spmd(_get_nc(), in_maps, core_ids=list(range(NCORES)), **kwargs)


def finish(results, b_out):
    y = np.zeros((T, DIM), np.float32)
    for r in results:
        y += np.asarray(r["out"], np.float32)
    y += np.asarray(b_out, np.float32)[None, :]
    return y.reshape(B, N, DIM).astype(np.float32)


def kernel(x, mask, attn_bias, w_qkv, w_out, b_out):
    in_maps = prepare_in_maps(x, mask, attn_bias, w_qkv, w_out, b_out)
    res = run_device(in_maps)
    return finish(res.results, b_out)
